# revision 24
# baseline (speedup 1.0000x reference)
"""Multi-head attention (B=2, S=2048, D=1024, H=16) on 8 TRN2 NeuronCores.

Sharding: 2-way data parallel over batch x 4-way tensor parallel over heads
(4 heads = 256 dims per core).  Each core computes, for its (batch, head
group): Q/K/V projections, causal attention, and a partial output
projection (row-sharded Wo).  The host sums the 4 fp16 partials per batch
and adds bo.

Schedule notes (v2 — built from the 210us baseline's neuron-profile):
  - The HAM clock gate runs the engines at 1.2GHz until the PE has been
    busy for a while, and re-throttles after every multi-us PE stall, so
    the whole design centers on keeping the PE continuously fed.
  - Inputs are repacked on the host into [Sq-chunk, D, 512] granules so
    the DMA stream (striped across the sync+gpsimd queues) arrives in
    exactly the order the projection matmuls consume it.  The projection
    loop is q-block/k-block/v-block per chunk, so the first matmul only
    needs Wq + the first two 256KB granule halves (~8us in).
  - Attention: scoresT[t, s] = kh @ qh^T per (pair, head); exp on ACT;
    AV matmuls are emitted TWO j-iterations behind their scores matmuls
    (one-behind still stalled the PE ~0.3us per j on the exp).  Scores
    PSUM tiles are per-head [128,512] from a 6-buf pool shared with the
    Wo-stage tiles (PSUM is only 8 banks: 6 shared + 2 psO).
  - Softmax: the AV matmul's ones-column gives the denominator in psO row
    64.  ACT copies those rows onto partitions {0,32,64,96} of one tile
    (no DMA in the chain), one reciprocal_approx_fast serves all 4 heads,
    and K=1 bf16 matmuls broadcast each reciprocal row across 64
    partitions; the normalize multiply reads that PSUM tile directly.
    The whole chain + Wo projection is deferred one chunk so the PE never
    waits on it; the last chunk uses per-pair chains to shorten the tail.
  - Y partials are written as fp16 (halves the output traffic; the host
    accumulates in fp32).
"""

import sys

sys.path.insert(0, "/opt/trn_rl_repo")

from contextlib import ExitStack

import ml_dtypes
import numpy as np

B, S, D, H = 2, 2048, 1024, 16
DK = D // H            # 64
NCORE = 8
DPB = 2                # data-parallel ways (batch)
TPG = NCORE // DPB     # 4 head groups
GH = H // TPG          # 4 heads per group
GD = GH * DK           # 256 dims per group
NPAIR = GH // 2        # 2 head pairs per group
SQC = 512              # Sq chunk (matmul moving dim)
SKC = 128              # Skv chunk (matmul partition dim)
MCH = 128              # mask chunk width
NI = S // SQC          # 4
NJ = S // SKC          # 16
KCH = D // 128         # 8 contraction chunks for the projections
HGD = KCH // 2 * GD    # weight-half column count (1024)

TRACE = False
LAST_EXEC_NS = None
LAST_RESULT = None

_BF = ml_dtypes.bfloat16
_prog_cache = {}


def _classify_mask(mask_st):
    """mask_st: [S, S] bool indexed [query s, key t].

    Returns (cls, tiles): cls[i][j] is None (skip) or a dict with
      lo, hi : active scoresT column range (multiples of MCH)
      muls   : list of (col_off, tile_idx) 128-col multiplicative masks
    tiles: deduped bf16 [SKC, MCH] tiles in scoresT orientation [t, s].
    """
    cls = [[None] * NJ for _ in range(NI)]
    tiles = []
    keys = {}

    def tile_idx(sub):
        t = np.ascontiguousarray(sub.T)  # [SKC t, MCH s]
        key = t.tobytes()
        if key not in keys:
            keys[key] = len(tiles)
            tiles.append(t.astype(_BF))
        return keys[key]

    for i in range(NI):
        sblk = mask_st[i * SQC : (i + 1) * SQC]
        for j in range(NJ):
            blk = sblk[:, j * SKC : (j + 1) * SKC]  # [SQC s, SKC t]
            any_col = blk.any(axis=1)               # per query col of scoresT
            if not any_col.any():
                continue
            nz = np.nonzero(any_col)[0]
            lo = (int(nz[0]) // MCH) * MCH
            hi = -(-(int(nz[-1]) + 1) // MCH) * MCH
            muls = []
            for c in range(lo, hi, MCH):
                sub = blk[c : c + MCH]              # [MCH s, SKC t]
                if not sub.all():
                    muls.append((c, tile_idx(sub)))
            cls[i][j] = {"lo": lo, "hi": hi, "muls": muls}
    return cls, tiles


def _build(cls, n_mask, with_bias):
    """Build the (SPMD, per-core) Bass program."""
    import concourse.bacc as bacc
    import concourse.tile as tile
    from concourse import mybir

    BF = mybir.dt.bfloat16
    F16 = mybir.dt.float16
    F32 = mybir.dt.float32
    AF = mybir.ActivationFunctionType

    nc = bacc.Bacc("TRN2", target_bir_lowering=False, debug=False)

    # x inputs repacked on host: [NI, D, SQC] -> rows sc*D+d, cols c
    xq_d = nc.dram_tensor("XQ", [NI * D, SQC], BF, kind="ExternalInput").ap()
    xk_d = nc.dram_tensor("XK", [NI * D, SQC], BF, kind="ExternalInput").ap()
    xv_d = nc.dram_tensor("XV", [NI * D, SQC], BF, kind="ExternalInput").ap()
    # packed weights: [128, KCH*GD], chunk kk at cols [kk*GD, (kk+1)*GD)
    wq_d = nc.dram_tensor("WQ", [128, KCH * GD], BF, kind="ExternalInput").ap()
    wk_d = nc.dram_tensor("WK", [128, KCH * GD], BF, kind="ExternalInput").ap()
    wv_d = nc.dram_tensor("WV", [128, KCH * GD], BF, kind="ExternalInput").ap()
    # packed Wo.T slice: [128, 2*D], chunk kc at cols [kc*D, (kc+1)*D)
    wo_d = nc.dram_tensor("WO", [128, 2 * D], BF, kind="ExternalInput").ap()
    msk_d = None
    if n_mask:
        msk_d = nc.dram_tensor(
            "MSK", [n_mask, SKC, MCH], BF, kind="ExternalInput"
        ).ap()
    if with_bias:
        bq_d = nc.dram_tensor("BQ", [1, GD], BF, kind="ExternalInput").ap()
        bk_d = nc.dram_tensor("BK", [1, GD], BF, kind="ExternalInput").ap()
        bv_d = nc.dram_tensor("BV", [1, GD], BF, kind="ExternalInput").ap()
    y_d = nc.dram_tensor("Y", [S, D], F16, kind="ExternalOutput").ap()

    with tile.TileContext(nc) as tc, ExitStack() as top:
        const = top.enter_context(tc.tile_pool(name="const", bufs=1))

        wq_sb = const.tile([128, KCH * GD], BF, name="wq_sb", tag="wq_sb")
        wk_sb = const.tile([128, KCH * GD], BF, name="wk_sb", tag="wk_sb")
        wv_sb = const.tile([128, KCH * GD], BF, name="wv_sb", tag="wv_sb")
        wo_sb = const.tile([128, 2 * D], BF, name="wo_sb", tag="wo_sb")
        # x tiles: xt[t][sc][kk] is [128, SQC] — one tile per contraction
        # chunk so each projection matmul waits only on its own 128KB DMA
        xt = {}
        for t in "qkv":
            xt[t] = [
                [
                    const.tile([128, SQC], BF, name=f"x{t}{sc}{kk}",
                               tag=f"x{t}{sc}{kk}")
                    for kk in range(KCH)
                ]
                for sc in range(NI)
            ]

        ENG = [nc.sync, nc.gpsimd]
        xdram = {"q": xq_d, "k": xk_d, "v": xv_d}

        def ldhalf(h, t, sc):
            # 4 contiguous [128, SQC] chunk transfers, one tile each
            for b in range(4):
                kk = h * 4 + b
                r0 = sc * D + kk * 128
                ENG[h].dma_start(
                    out=xt[t][sc][kk][:],
                    in_=xdram[t][r0 : r0 + 128, :],
                )

        # DMA stream order == projection consumption order, halves striped
        # across the two queues.
        for h in range(2):
            ENG[h].dma_start(
                out=wq_sb[:, h * HGD : (h + 1) * HGD],
                in_=wq_d[:, h * HGD : (h + 1) * HGD],
            )
        for h in range(2):
            ldhalf(h, "q", 0)
        for h in range(2):
            ENG[h].dma_start(
                out=wk_sb[:, h * HGD : (h + 1) * HGD],
                in_=wk_d[:, h * HGD : (h + 1) * HGD],
            )
        for h in range(2):
            ldhalf(h, "k", 0)
        for h in range(2):
            ENG[h].dma_start(
                out=wv_sb[:, h * HGD : (h + 1) * HGD],
                in_=wv_d[:, h * HGD : (h + 1) * HGD],
            )
        if with_bias:
            bq_sb = const.tile([1, GD], BF, name="bq_sb", tag="bq_sb")
            bk_sb = const.tile([1, GD], BF, name="bk_sb", tag="bk_sb")
            bv_sb = const.tile([1, GD], BF, name="bv_sb", tag="bv_sb")
            nc.sync.dma_start(out=bq_sb[:], in_=bq_d[:])
            nc.sync.dma_start(out=bk_sb[:], in_=bk_d[:])
            nc.sync.dma_start(out=bv_sb[:], in_=bv_d[:])
        for h in range(2):
            ldhalf(h, "v", 0)
        for sc in range(1, NI):
            for t in "qkv":
                for h in range(2):
                    ldhalf(h, t, sc)
            if sc == 1:
                msk_sb = []
                for mt in range(n_mask):
                    m = const.tile([SKC, MCH], BF, name=f"msk{mt}",
                                   tag=f"msk{mt}")
                    nc.sync.dma_start(out=m[:], in_=msk_d[mt])
                    msk_sb.append(m)
        if n_mask == 0:
            msk_sb = []
        for h in range(2):
            ENG[h].dma_start(
                out=wo_sb[:, h * D : (h + 1) * D],
                in_=wo_d[:, h * D : (h + 1) * D],
            )

        if with_bias:
            onesrow = const.tile([1, SQC], BF, name="onesrow", tag="onesrow")
            nc.vector.memset(onesrow[:], 1.0)

        # warm up the ACT exp table during the projection phase so the
        # 1.3us ACT_TABLE_LOAD isn't on the first attention exp
        warm0 = const.tile([1, 64], BF, name="warm0", tag="warm0")
        warm1 = const.tile([1, 64], BF, name="warm1", tag="warm1")
        nc.vector.memset(warm0[:], 0.0)
        nc.scalar.activation(warm1[:], warm0[:], AF.Exp)

        # persistent activations
        acts = top.enter_context(tc.tile_pool(name="acts", bufs=1))
        qhT = [acts.tile([128, S], BF, name=f"qhT{p}", tag=f"qhT{p}")
               for p in range(NPAIR)]
        khT = [acts.tile([128, S], BF, name=f"khT{p}", tag=f"khT{p}")
               for p in range(NPAIR)]
        # v in natural layout, 65 cols per head (64 dims + ones column)
        vh = [acts.tile([128, GH * 65], BF, name=f"vh{j}", tag=f"vh{j}")
              for j in range(NJ)]
        for j in range(NJ):
            v3 = vh[j].rearrange("p (h x) -> p h x", h=GH)
            nc.vector.memset(v3[:, :, 64:65], 1.0)

        def xsl(t, sc, kk):
            return xt[t][sc][kk][:]

        # ---------------- Phase B: projections ----------------
        with (
            tc.tile_pool(name="pproj", bufs=1, space="PSUM") as pproj,
        ):
            for sc in range(NI):
                psq = [pproj.tile([128, SQC], F32, name=f"psq{m}", tag=f"psq{m}")
                       for m in range(2)]
                psk = [pproj.tile([128, SQC], F32, name=f"psk{m}", tag=f"psk{m}")
                       for m in range(2)]
                psv = [pproj.tile([128, GD], F32, name=f"psv{m}", tag=f"psv{m}")
                       for m in range(4)]
                cc = slice(sc * SQC, (sc + 1) * SQC)
                # q block
                for kk in range(KCH):
                    st = kk == 0
                    sp = (kk == KCH - 1) and not with_bias
                    for m in range(2):
                        wcol = slice(kk * GD + m * 128, kk * GD + (m + 1) * 128)
                        nc.tensor.matmul(
                            psq[m][:], wq_sb[:, wcol], xsl("q", sc, kk),
                            start=st, stop=sp,
                        )
                if with_bias:
                    for m in range(2):
                        bcol = slice(m * 128, (m + 1) * 128)
                        nc.tensor.matmul(
                            psq[m][:], bq_sb[:, bcol], onesrow[:],
                            start=False, stop=True,
                        )
                for m in range(2):
                    nc.vector.tensor_copy(qhT[m][:, cc], psq[m][:])
                # k block
                for kk in range(KCH):
                    st = kk == 0
                    sp = (kk == KCH - 1) and not with_bias
                    for m in range(2):
                        wcol = slice(kk * GD + m * 128, kk * GD + (m + 1) * 128)
                        nc.tensor.matmul(
                            psk[m][:], wk_sb[:, wcol], xsl("k", sc, kk),
                            start=st, stop=sp,
                        )
                if with_bias:
                    for m in range(2):
                        bcol = slice(m * 128, (m + 1) * 128)
                        nc.tensor.matmul(
                            psk[m][:], bk_sb[:, bcol], onesrow[:],
                            start=False, stop=True,
                        )
                for m in range(2):
                    nc.vector.tensor_copy(khT[m][:, cc], psk[m][:])
                # v block
                for kk in range(KCH):
                    st = kk == 0
                    sp = (kk == KCH - 1) and not with_bias
                    for m in range(4):
                        nc.tensor.matmul(
                            psv[m][:],
                            xsl("v", sc, kk)[:, m * 128 : (m + 1) * 128],
                            wv_sb[:, kk * GD : (kk + 1) * GD],
                            start=st,
                            stop=sp,
                        )
                if with_bias:
                    for m in range(4):
                        nc.tensor.matmul(
                            psv[m][:], onesrow[:, 0:128], bv_sb[:],
                            start=False, stop=True,
                        )
                for m in range(4):
                    dst = vh[sc * 4 + m].rearrange("p (h x) -> p h x", h=GH)
                    src = psv[m].rearrange("p (h x) -> p h x", h=GH)
                    nc.vector.tensor_copy(dst[:, :, 0:64], src[:])

        # ---------------- Phase C: attention + Wo ----------------
        # ones on all 128 partitions; single rows are the lhsT of the K=1
        # denominator-broadcast matmuls (lhsT base must match rhs row base)
        onesP = const.tile([128, 64], BF, name="onesP", tag="onesP")
        nc.vector.memset(onesP[:], 1.0)

        with (
            # 3 two-bank bufs shared by the scores tiles and the Wo-stage
            # psB/pY tiles; +2 banks for psO = all 8 PSUM banks
            tc.tile_pool(name="psh", bufs=3, space="PSUM") as psh,
            tc.tile_pool(name="pso", bufs=1, space="PSUM") as pso,
            tc.tile_pool(name="ex", bufs=4) as expool,
            tc.tile_pool(name="nrm", bufs=2) as nrm,
            tc.tile_pool(name="aou", bufs=8) as aoupool,
            tc.tile_pool(name="ao", bufs=2) as aopool,
            tc.tile_pool(name="yout", bufs=3) as ypool,
        ):
            def emit_bcast_norm(state):
                """K=1 broadcast matmuls + normalize muls for a finished i;
                emitted between the two pairs of the NEXT chunk's attention
                so the Wo matmuls find aoT ready."""
                aoT, aoUs, rcb_info = (
                    state["aoT"], state["aoUs"], state["rcb_info"]
                )
                for idx in range(4):
                    rcb_t, r = rcb_info[idx]
                    p, h = divmod(idx, 2)
                    psB = psh.tile([64, SQC], F32, name="psB", tag="psh")
                    nc.tensor.matmul(
                        psB[:],
                        onesP[r : r + 1, :],
                        rcb_t[r : r + 1, :],
                        start=True, stop=True,
                        tile_position=(r, 0),
                    )
                    # normalize reads the PSUM broadcast tile directly
                    nc.vector.tensor_mul(
                        aoT[p][h * 64 : (h + 1) * 64, :],
                        aoUs[idx][0:64, :],
                        psB[:],
                    )

            def emit_attention(i, pending):
                """scores/exp/mask/AV + psO evacuation + reciprocal chain."""
                js = [j for j in range(NJ) if cls[i][j] is not None]
                assert js, "fully-masked query chunk not supported"
                aoT = [
                    aopool.tile([128, SQC], BF, name=f"aoT{p}", tag=f"aoT{p}")
                    for p in range(NPAIR)
                ]
                if i < NI - 1:
                    den_t = nrm.tile([97, SQC], F32, name="den_t", tag="den_t")
                    nc.gpsimd.memset(den_t[:], 1.0)
                aoUs = []
                rcbs = []
                for p in range(NPAIR):
                    psO = [
                        pso.tile([65, SQC], F32, name=f"psO{h}", tag=f"psO{h}")
                        for h in range(2)
                    ]

                    def emit_av(av):
                        jn, j, lo, hi, e = av
                        for h in range(2):
                            vcol = slice((2 * p + h) * 65, (2 * p + h + 1) * 65)
                            nc.tensor.matmul(
                                psO[h][:, lo:hi],
                                vh[j][:, vcol],
                                e[:, h * SQC + lo : h * SQC + hi],
                                start=(jn == 0), stop=(jn == len(js) - 1),
                            )

                    # AV matmuls are emitted TWO j-iterations behind the
                    # scores matmuls so the in-order PE never waits on the
                    # ACT engine's exp.
                    pend = []
                    for jn, j in enumerate(js):
                        c = cls[i][j]
                        lo, hi = c["lo"], c["hi"]
                        jw = slice(j * SKC, (j + 1) * SKC)
                        iw = slice(i * SQC + lo, i * SQC + hi)
                        # h0 in cols [0:SQC], h1 in cols [SQC:2*SQC]
                        ps = psh.tile([128, 2 * SQC], F32, name="ps", tag="psh")
                        e = expool.tile([128, 2 * SQC], BF, name="e", tag="ex")
                        for h in range(2):
                            pr = slice(h * 64, (h + 1) * 64)
                            nc.tensor.matmul(
                                ps[:, h * SQC + lo : h * SQC + hi],
                                khT[p][pr, jw],
                                qhT[p][pr, iw],
                                start=True, stop=True,
                            )
                        ps3 = ps.rearrange("p (h c) -> p h c", h=2)
                        e3 = e.rearrange("p (h c) -> p h c", h=2)
                        nc.scalar.activation(
                            e3[:, :, lo:hi], ps3[:, :, lo:hi], AF.Exp,
                            scale=1.0 / np.sqrt(DK),
                        )
                        for c0, tidx in c["muls"]:
                            for h in range(2):
                                cw = slice(h * SQC + c0, h * SQC + c0 + MCH)
                                nc.vector.tensor_mul(
                                    e[:, cw], e[:, cw], msk_sb[tidx][:]
                                )
                        pend.append((jn, j, lo, hi, e))
                        if len(pend) > 2:
                            emit_av(pend.pop(0))
                    for av in pend:
                        emit_av(av)
                    # aoU evacuation (incl. denominator row 64) frees psO
                    # for the next pair; ACT gathers the denominator rows
                    # from SBUF (no DMA in the chain)
                    paoU = []
                    for h in range(2):
                        aoU = aoupool.tile([65, SQC], F32, name="aoU",
                                           tag="aoU")
                        nc.vector.tensor_copy(aoU[:], psO[h][:])
                        aoUs.append(aoU)
                        paoU.append(aoU)
                    if i < NI - 1:
                        for h in range(2):
                            idx = 2 * p + h
                            nc.scalar.copy(
                                den_t[32 * idx : 32 * idx + 1, :],
                                paoU[h][64:65, :],
                            )
                    else:
                        # last chunk: per-pair reciprocal so pair 0's chain
                        # hides under pair 1's attention instead of tailing
                        den_p = nrm.tile([33, SQC], F32, name="den_p",
                                         tag="den_p")
                        nc.gpsimd.memset(den_p[:], 1.0)
                        for h in range(2):
                            nc.scalar.copy(
                                den_p[32 * h : 32 * h + 1, :],
                                paoU[h][64:65, :],
                            )
                        rc_p = nrm.tile([33, SQC], F32, name="rc_p", tag="rc_p")
                        nc.vector.reciprocal_approx_fast(rc_p[:], den_p[:])
                        rcb_p = nrm.tile([33, SQC], BF, name="rcb_p",
                                         tag="rcb_p")
                        nc.vector.tensor_copy(rcb_p[:], rc_p[:])
                        rcbs.append(rcb_p)
                    # deferred broadcast+normalize of the previous chunk,
                    # hidden under this chunk's second pair
                    if p == 0 and pending is not None:
                        emit_bcast_norm(pending)
                if i == NI - 1:
                    rcb_info = [(rcbs[0], 0), (rcbs[0], 32),
                                (rcbs[1], 0), (rcbs[1], 32)]
                    return {"i": i, "aoT": aoT, "aoUs": aoUs,
                            "rcb_info": rcb_info}
                return {"i": i, "aoT": aoT, "aoUs": aoUs, "den_t": den_t,
                        "rcb_info": None}

            def emit_recip(state):
                """One reciprocal serves all 4 heads (rows {0,32,64,96}).
                Emitted after the previous chunk's y casts so the DVE
                serves those first; only needed at the next pair boundary."""
                den_t = state["den_t"]
                rc_t = nrm.tile([97, SQC], F32, name="rc_t", tag="rc_t")
                nc.vector.reciprocal_approx_fast(rc_t[:], den_t[:])
                rcb_t = nrm.tile([97, SQC], BF, name="rcb_t", tag="rcb_t")
                nc.vector.tensor_copy(rcb_t[:], rc_t[:])
                state["rcb_info"] = [(rcb_t, 0), (rcb_t, 32), (rcb_t, 64),
                                     (rcb_t, 96)]

            def emit_norm_wo(state):
                """Wo projection + Y write for a finished i (deferred one
                chunk so the PE never waits on the reciprocal chain)."""
                i, aoT = state["i"], state["aoT"]
                for m in range(4):
                    rw = slice(m * 128, (m + 1) * 128)
                    y_sb = ypool.tile([128, D], F16, name="y_sb", tag="y_sb")
                    for n in range(2):
                        pY = psh.tile([128, SQC], F32, name="pY", tag="psh")
                        for kc in range(NPAIR):
                            nc.tensor.matmul(
                                pY[:],
                                aoT[kc][:, rw],
                                wo_sb[:, kc * D + n * SQC : kc * D + (n + 1) * SQC],
                                start=(kc == 0),
                                stop=(kc == NPAIR - 1),
                            )
                        nc.vector.tensor_copy(
                            y_sb[:, n * SQC : (n + 1) * SQC], pY[:]
                        )
                    # full 2KB dram rows per DMA, alternating queues
                    ENG[m % 2].dma_start(
                        out=y_d[i * SQC + m * 128 : i * SQC + (m + 1) * 128, :],
                        in_=y_sb[:],
                    )

            pending = None
            for i in range(NI):
                st = emit_attention(i, pending)
                if pending is not None:
                    emit_norm_wo(pending)
                if i < NI - 1:
                    emit_recip(st)
                pending = st
            emit_bcast_norm(pending)
            emit_norm_wo(pending)

    nc.compile()
    return nc


def _cls_sig(cls):
    out = []
    for row in cls:
        for c in row:
            if c is None:
                out.append(None)
            else:
                out.append((c["lo"], c["hi"], tuple(c["muls"])))
    return tuple(out)


def kernel(q, k, v, Wq, bq, Wk, bk, Wv, bv, Wo, bo, mask):
    global LAST_EXEC_NS, LAST_RESULT
    from concourse.bass_utils import run_bass_kernel_spmd

    q = np.asarray(q, np.float32)
    k = np.asarray(k, np.float32)
    v = np.asarray(v, np.float32)
    mask_st = np.asarray(mask).reshape(S, S).astype(bool)

    cls, mtiles = _classify_mask(mask_st)
    with_bias = not (
        np.all(np.asarray(bq) == 0)
        and np.all(np.asarray(bk) == 0)
        and np.all(np.asarray(bv) == 0)
    )

    sig = (_cls_sig(cls), len(mtiles), with_bias)
    if sig not in _prog_cache:
        _prog_cache[sig] = _build(cls, len(mtiles), with_bias)
    nc = _prog_cache[sig]

    def pack_w(wt, gd):  # [nch*128, gd] -> [128, nch*gd]
        nch = wt.shape[0] // 128
        return np.ascontiguousarray(
            wt.reshape(nch, 128, gd).transpose(1, 0, 2).reshape(128, nch * gd)
        ).astype(_BF)

    def pack_x(xb):  # [S, D] -> [NI*D, SQC]  (xT column-granules)
        xT = xb.T  # [D, S]
        return np.ascontiguousarray(
            xT.reshape(D, NI, SQC).transpose(1, 0, 2).reshape(NI * D, SQC)
        ).astype(_BF)

    xq_p = [pack_x(q[b]) for b in range(B)]
    xk_p = [pack_x(k[b]) for b in range(B)]
    xv_p = [pack_x(v[b]) for b in range(B)]

    in_maps = []
    for c in range(NCORE):
        b, g = divmod(c, TPG)
        rows = slice(g * GD, (g + 1) * GD)
        im = {
            "XQ": xq_p[b],
            "XK": xk_p[b],
            "XV": xv_p[b],
            "WQ": pack_w(np.ascontiguousarray(Wq[rows, :].T), GD),
            "WK": pack_w(np.ascontiguousarray(Wk[rows, :].T), GD),
            "WV": pack_w(np.ascontiguousarray(Wv[rows, :].T), GD),
            "WO": pack_w(np.ascontiguousarray(Wo[:, rows].T), D),
        }
        if mtiles:
            im["MSK"] = np.stack(mtiles)
        if with_bias:
            im["BQ"] = np.asarray(bq)[rows].reshape(1, GD).astype(_BF)
            im["BK"] = np.asarray(bk)[rows].reshape(1, GD).astype(_BF)
            im["BV"] = np.asarray(bv)[rows].reshape(1, GD).astype(_BF)
        in_maps.append(im)

    res = run_bass_kernel_spmd(nc, in_maps, list(range(NCORE)), trace=TRACE)
    LAST_RESULT = res
    LAST_EXEC_NS = res.exec_time_ns

    out = np.zeros((B, S, D), np.float32)
    for c in range(NCORE):
        out[c // TPG] += res.results[c]["Y"].astype(np.float32)
    out += np.asarray(bo, np.float32)
    return out


# revision 25
# speedup vs baseline: 1.0328x; 1.0328x over previous
"""Multi-head attention (B=2, S=2048, D=1024, H=16) on 8 TRN2 NeuronCores.

Sharding: 2-way data parallel over batch x 4-way tensor parallel over heads
(4 heads = 256 dims per core).  Each core computes, for its (batch, head
group): Q/K/V projections, causal attention, and a partial output
projection (row-sharded Wo).  The host sums the 4 fp16 partials per batch
and adds bo.

Schedule notes (built from neuron-profile traces of each revision; the
baseline this evolved from ran 210us, this version ~195us):
  - The HAM power governor starts the clocks at 1.2GHz and claws back
    full-rate (2.4GHz) windows in proportion to total engine activity,
    so the design minimizes total engine busy-time as much as stalls:
    merged exp instructions, few large DMAs, approximate reciprocal.
  - Inputs are repacked on the host into [Sq-chunk, D, 512] granules so
    the DMA stream (striped across the sync+gpsimd queues) arrives in
    exactly the order the projection matmuls consume it, one 128KB tile
    per contraction chunk.  The projection loop is q-block/k-block/
    v-block per Sq chunk; the phase is PE-bound, never DMA-starved.
  - Attention is co-limited by the PE and the ACT engine's exp (their
    per-chunk work is almost exactly equal).  scoresT[t, s] = kh @ qh^T
    per (pair, head) in one [128, 2*512] PSUM tile; one exp instruction
    covers both heads; AV matmuls are emitted TWO j-iterations behind
    their scores matmuls so the in-order PE rides out exp latency.
    PSUM budget: 3x2-bank scores bufs shared with the Wo-stage psB/pY
    tiles + 2 banks psO = all 8 banks.
  - Softmax: the AV matmul's ones-column gives the denominator in psO
    row 64; it is evacuated with the attention numerators in one DVE
    copy, ACT gathers the 4 rows onto partitions {0,32,64,96} of one
    tile (no DMA in the chain), one reciprocal_approx_fast serves all
    4 heads, and K=1 bf16 matmuls broadcast each reciprocal row across
    64 partitions; the normalize multiply reads that PSUM tile
    directly.  The chain is deferred one chunk: its broadcast+normalize
    hides under the next chunk's second pair, the Wo projection runs
    after that chunk, and the reciprocal is emitted behind the Y casts
    so the DVE serves the Wo stage first.  The last chunk uses per-pair
    chains to shorten the tail; the ACT exp table is pre-warmed during
    the projections.
  - Y partials are written as fp16 (halves the output traffic; the host
    accumulates in fp32; fp16 keeps 10 mantissa bits so the partial-sum
    rounding is negligible).
"""

import sys

sys.path.insert(0, "/opt/trn_rl_repo")

from contextlib import ExitStack

import ml_dtypes
import numpy as np

B, S, D, H = 2, 2048, 1024, 16
DK = D // H            # 64
NCORE = 8
DPB = 2                # data-parallel ways (batch)
TPG = NCORE // DPB     # 4 head groups
GH = H // TPG          # 4 heads per group
GD = GH * DK           # 256 dims per group
NPAIR = GH // 2        # 2 head pairs per group
SQC = 512              # Sq chunk (matmul moving dim)
SKC = 128              # Skv chunk (matmul partition dim)
MCH = 128              # mask chunk width
NI = S // SQC          # 4
NJ = S // SKC          # 16
KCH = D // 128         # 8 contraction chunks for the projections
HGD = KCH // 2 * GD    # weight-half column count (1024)

TRACE = False
LAST_EXEC_NS = None
LAST_RESULT = None

_BF = ml_dtypes.bfloat16
_prog_cache = {}


def _classify_mask(mask_st):
    """mask_st: [S, S] bool indexed [query s, key t].

    Returns (cls, tiles): cls[i][j] is None (skip) or a dict with
      lo, hi : active scoresT column range (multiples of MCH)
      muls   : list of (col_off, tile_idx) 128-col multiplicative masks
    tiles: deduped bf16 [SKC, MCH] tiles in scoresT orientation [t, s].
    """
    cls = [[None] * NJ for _ in range(NI)]
    tiles = []
    keys = {}

    def tile_idx(sub):
        t = np.ascontiguousarray(sub.T)  # [SKC t, MCH s]
        key = t.tobytes()
        if key not in keys:
            keys[key] = len(tiles)
            tiles.append(t.astype(_BF))
        return keys[key]

    for i in range(NI):
        sblk = mask_st[i * SQC : (i + 1) * SQC]
        for j in range(NJ):
            blk = sblk[:, j * SKC : (j + 1) * SKC]  # [SQC s, SKC t]
            any_col = blk.any(axis=1)               # per query col of scoresT
            if not any_col.any():
                continue
            nz = np.nonzero(any_col)[0]
            lo = (int(nz[0]) // MCH) * MCH
            hi = -(-(int(nz[-1]) + 1) // MCH) * MCH
            muls = []
            for c in range(lo, hi, MCH):
                sub = blk[c : c + MCH]              # [MCH s, SKC t]
                if not sub.all():
                    muls.append((c, tile_idx(sub)))
            cls[i][j] = {"lo": lo, "hi": hi, "muls": muls}
    return cls, tiles


def _build(cls, n_mask, with_bias):
    """Build the (SPMD, per-core) Bass program."""
    import concourse.bacc as bacc
    import concourse.tile as tile
    from concourse import mybir

    BF = mybir.dt.bfloat16
    F16 = mybir.dt.float16
    F32 = mybir.dt.float32
    AF = mybir.ActivationFunctionType

    nc = bacc.Bacc("TRN2", target_bir_lowering=False, debug=False)

    # x inputs repacked on host: [NI, D, SQC] -> rows sc*D+d, cols c
    xq_d = nc.dram_tensor("XQ", [NI * D, SQC], BF, kind="ExternalInput").ap()
    xk_d = nc.dram_tensor("XK", [NI * D, SQC], BF, kind="ExternalInput").ap()
    xv_d = nc.dram_tensor("XV", [NI * D, SQC], BF, kind="ExternalInput").ap()
    # packed weights: [128, KCH*GD], chunk kk at cols [kk*GD, (kk+1)*GD)
    wq_d = nc.dram_tensor("WQ", [128, KCH * GD], BF, kind="ExternalInput").ap()
    wk_d = nc.dram_tensor("WK", [128, KCH * GD], BF, kind="ExternalInput").ap()
    wv_d = nc.dram_tensor("WV", [128, KCH * GD], BF, kind="ExternalInput").ap()
    # packed Wo.T slice: [128, 2*D], chunk kc at cols [kc*D, (kc+1)*D)
    wo_d = nc.dram_tensor("WO", [128, 2 * D], BF, kind="ExternalInput").ap()
    msk_d = None
    if n_mask:
        msk_d = nc.dram_tensor(
            "MSK", [n_mask, SKC, MCH], BF, kind="ExternalInput"
        ).ap()
    if with_bias:
        bq_d = nc.dram_tensor("BQ", [1, GD], BF, kind="ExternalInput").ap()
        bk_d = nc.dram_tensor("BK", [1, GD], BF, kind="ExternalInput").ap()
        bv_d = nc.dram_tensor("BV", [1, GD], BF, kind="ExternalInput").ap()
    y_d = nc.dram_tensor("Y", [S, D], F16, kind="ExternalOutput").ap()

    with tile.TileContext(nc) as tc, ExitStack() as top:
        const = top.enter_context(tc.tile_pool(name="const", bufs=1))

        wq_sb = const.tile([128, KCH * GD], BF, name="wq_sb", tag="wq_sb")
        wk_sb = const.tile([128, KCH * GD], BF, name="wk_sb", tag="wk_sb")
        wv_sb = const.tile([128, KCH * GD], BF, name="wv_sb", tag="wv_sb")
        wo_sb = const.tile([128, 2 * D], BF, name="wo_sb", tag="wo_sb")
        # x tiles: xt[t][sc][kk] is [128, SQC] — one tile per contraction
        # chunk so each projection matmul waits only on its own 128KB DMA
        xt = {}
        for t in "qkv":
            xt[t] = [
                [
                    const.tile([128, SQC], BF, name=f"x{t}{sc}{kk}",
                               tag=f"x{t}{sc}{kk}")
                    for kk in range(KCH)
                ]
                for sc in range(NI)
            ]

        ENG = [nc.sync, nc.gpsimd]
        xdram = {"q": xq_d, "k": xk_d, "v": xv_d}

        def ldhalf(h, t, sc):
            # 4 contiguous [128, SQC] chunk transfers, one tile each
            for b in range(4):
                kk = h * 4 + b
                r0 = sc * D + kk * 128
                ENG[h].dma_start(
                    out=xt[t][sc][kk][:],
                    in_=xdram[t][r0 : r0 + 128, :],
                )

        # DMA stream order == projection consumption order, halves striped
        # across the two queues.
        for h in range(2):
            ENG[h].dma_start(
                out=wq_sb[:, h * HGD : (h + 1) * HGD],
                in_=wq_d[:, h * HGD : (h + 1) * HGD],
            )
        for h in range(2):
            ldhalf(h, "q", 0)
        for h in range(2):
            ENG[h].dma_start(
                out=wk_sb[:, h * HGD : (h + 1) * HGD],
                in_=wk_d[:, h * HGD : (h + 1) * HGD],
            )
        for h in range(2):
            ldhalf(h, "k", 0)
        for h in range(2):
            ENG[h].dma_start(
                out=wv_sb[:, h * HGD : (h + 1) * HGD],
                in_=wv_d[:, h * HGD : (h + 1) * HGD],
            )
        if with_bias:
            bq_sb = const.tile([1, GD], BF, name="bq_sb", tag="bq_sb")
            bk_sb = const.tile([1, GD], BF, name="bk_sb", tag="bk_sb")
            bv_sb = const.tile([1, GD], BF, name="bv_sb", tag="bv_sb")
            nc.sync.dma_start(out=bq_sb[:], in_=bq_d[:])
            nc.sync.dma_start(out=bk_sb[:], in_=bk_d[:])
            nc.sync.dma_start(out=bv_sb[:], in_=bv_d[:])
        for h in range(2):
            ldhalf(h, "v", 0)
        for sc in range(1, NI):
            for t in "qkv":
                for h in range(2):
                    ldhalf(h, t, sc)
            if sc == 1:
                msk_sb = []
                for mt in range(n_mask):
                    m = const.tile([SKC, MCH], BF, name=f"msk{mt}",
                                   tag=f"msk{mt}")
                    nc.sync.dma_start(out=m[:], in_=msk_d[mt])
                    msk_sb.append(m)
        if n_mask == 0:
            msk_sb = []
        for h in range(2):
            ENG[h].dma_start(
                out=wo_sb[:, h * D : (h + 1) * D],
                in_=wo_d[:, h * D : (h + 1) * D],
            )

        if with_bias:
            onesrow = const.tile([1, SQC], BF, name="onesrow", tag="onesrow")
            nc.vector.memset(onesrow[:], 1.0)

        # warm up the ACT exp table during the projection phase so the
        # 1.3us ACT_TABLE_LOAD isn't on the first attention exp
        warm0 = const.tile([1, 64], BF, name="warm0", tag="warm0")
        warm1 = const.tile([1, 64], BF, name="warm1", tag="warm1")
        nc.vector.memset(warm0[:], 0.0)
        nc.scalar.activation(warm1[:], warm0[:], AF.Exp)

        # persistent activations
        acts = top.enter_context(tc.tile_pool(name="acts", bufs=1))
        qhT = [acts.tile([128, S], BF, name=f"qhT{p}", tag=f"qhT{p}")
               for p in range(NPAIR)]
        khT = [acts.tile([128, S], BF, name=f"khT{p}", tag=f"khT{p}")
               for p in range(NPAIR)]
        # v in natural layout, 65 cols per head (64 dims + ones column)
        vh = [acts.tile([128, GH * 65], BF, name=f"vh{j}", tag=f"vh{j}")
              for j in range(NJ)]
        for j in range(NJ):
            v3 = vh[j].rearrange("p (h x) -> p h x", h=GH)
            nc.vector.memset(v3[:, :, 64:65], 1.0)

        def xsl(t, sc, kk):
            return xt[t][sc][kk][:]

        # ---------------- Phase B: projections ----------------
        with (
            tc.tile_pool(name="pproj", bufs=1, space="PSUM") as pproj,
        ):
            for sc in range(NI):
                psq = [pproj.tile([128, SQC], F32, name=f"psq{m}", tag=f"psq{m}")
                       for m in range(2)]
                psk = [pproj.tile([128, SQC], F32, name=f"psk{m}", tag=f"psk{m}")
                       for m in range(2)]
                psv = [pproj.tile([128, GD], F32, name=f"psv{m}", tag=f"psv{m}")
                       for m in range(4)]
                cc = slice(sc * SQC, (sc + 1) * SQC)
                # q block
                for kk in range(KCH):
                    st = kk == 0
                    sp = (kk == KCH - 1) and not with_bias
                    for m in range(2):
                        wcol = slice(kk * GD + m * 128, kk * GD + (m + 1) * 128)
                        nc.tensor.matmul(
                            psq[m][:], wq_sb[:, wcol], xsl("q", sc, kk),
                            start=st, stop=sp,
                        )
                if with_bias:
                    for m in range(2):
                        bcol = slice(m * 128, (m + 1) * 128)
                        nc.tensor.matmul(
                            psq[m][:], bq_sb[:, bcol], onesrow[:],
                            start=False, stop=True,
                        )
                for m in range(2):
                    nc.vector.tensor_copy(qhT[m][:, cc], psq[m][:])
                # k block
                for kk in range(KCH):
                    st = kk == 0
                    sp = (kk == KCH - 1) and not with_bias
                    for m in range(2):
                        wcol = slice(kk * GD + m * 128, kk * GD + (m + 1) * 128)
                        nc.tensor.matmul(
                            psk[m][:], wk_sb[:, wcol], xsl("k", sc, kk),
                            start=st, stop=sp,
                        )
                if with_bias:
                    for m in range(2):
                        bcol = slice(m * 128, (m + 1) * 128)
                        nc.tensor.matmul(
                            psk[m][:], bk_sb[:, bcol], onesrow[:],
                            start=False, stop=True,
                        )
                for m in range(2):
                    nc.vector.tensor_copy(khT[m][:, cc], psk[m][:])
                # v block
                for kk in range(KCH):
                    st = kk == 0
                    sp = (kk == KCH - 1) and not with_bias
                    for m in range(4):
                        nc.tensor.matmul(
                            psv[m][:],
                            xsl("v", sc, kk)[:, m * 128 : (m + 1) * 128],
                            wv_sb[:, kk * GD : (kk + 1) * GD],
                            start=st,
                            stop=sp,
                        )
                if with_bias:
                    for m in range(4):
                        nc.tensor.matmul(
                            psv[m][:], onesrow[:, 0:128], bv_sb[:],
                            start=False, stop=True,
                        )
                for m in range(4):
                    dst = vh[sc * 4 + m].rearrange("p (h x) -> p h x", h=GH)
                    src = psv[m].rearrange("p (h x) -> p h x", h=GH)
                    nc.vector.tensor_copy(dst[:, :, 0:64], src[:])

        # ---------------- Phase C: attention + Wo ----------------
        # ones on all 128 partitions; single rows are the lhsT of the K=1
        # denominator-broadcast matmuls (lhsT base must match rhs row base)
        onesP = const.tile([128, 64], BF, name="onesP", tag="onesP")
        nc.vector.memset(onesP[:], 1.0)

        with (
            # 3 two-bank bufs shared by the scores tiles and the Wo-stage
            # psB/pY tiles; +2 banks for psO = all 8 PSUM banks
            tc.tile_pool(name="psh", bufs=3, space="PSUM") as psh,
            tc.tile_pool(name="pso", bufs=1, space="PSUM") as pso,
            tc.tile_pool(name="ex", bufs=4) as expool,
            tc.tile_pool(name="nrm", bufs=2) as nrm,
            tc.tile_pool(name="aou", bufs=8) as aoupool,
            tc.tile_pool(name="ao", bufs=2) as aopool,
            tc.tile_pool(name="yout", bufs=3) as ypool,
        ):
            def emit_bcast_norm(state):
                """K=1 broadcast matmuls + normalize muls for a finished i;
                emitted between the two pairs of the NEXT chunk's attention
                so the Wo matmuls find aoT ready."""
                aoT, aoUs, rcb_info = (
                    state["aoT"], state["aoUs"], state["rcb_info"]
                )
                for idx in range(4):
                    rcb_t, r = rcb_info[idx]
                    p, h = divmod(idx, 2)
                    psB = psh.tile([64, SQC], F32, name="psB", tag="psh")
                    nc.tensor.matmul(
                        psB[:],
                        onesP[r : r + 1, :],
                        rcb_t[r : r + 1, :],
                        start=True, stop=True,
                        tile_position=(r, 0),
                    )
                    # normalize reads the PSUM broadcast tile directly
                    nc.vector.tensor_mul(
                        aoT[p][h * 64 : (h + 1) * 64, :],
                        aoUs[idx][0:64, :],
                        psB[:],
                    )

            def emit_attention(i, pending):
                """scores/exp/mask/AV + psO evacuation + reciprocal chain."""
                js = [j for j in range(NJ) if cls[i][j] is not None]
                assert js, "fully-masked query chunk not supported"
                aoT = [
                    aopool.tile([128, SQC], BF, name=f"aoT{p}", tag=f"aoT{p}")
                    for p in range(NPAIR)
                ]
                if i < NI - 1:
                    den_t = nrm.tile([97, SQC], F32, name="den_t", tag="den_t")
                    nc.gpsimd.memset(den_t[:], 1.0)
                aoUs = []
                rcbs = []
                for p in range(NPAIR):
                    psO = [
                        pso.tile([65, SQC], F32, name=f"psO{h}", tag=f"psO{h}")
                        for h in range(2)
                    ]

                    def emit_av(av):
                        jn, j, lo, hi, e = av
                        for h in range(2):
                            vcol = slice((2 * p + h) * 65, (2 * p + h + 1) * 65)
                            nc.tensor.matmul(
                                psO[h][:, lo:hi],
                                vh[j][:, vcol],
                                e[:, h * SQC + lo : h * SQC + hi],
                                start=(jn == 0), stop=(jn == len(js) - 1),
                            )

                    # AV matmuls are emitted TWO j-iterations behind the
                    # scores matmuls so the in-order PE never waits on the
                    # ACT engine's exp.
                    pend = []
                    for jn, j in enumerate(js):
                        c = cls[i][j]
                        lo, hi = c["lo"], c["hi"]
                        jw = slice(j * SKC, (j + 1) * SKC)
                        iw = slice(i * SQC + lo, i * SQC + hi)
                        # h0 in cols [0:SQC], h1 in cols [SQC:2*SQC]
                        ps = psh.tile([128, 2 * SQC], F32, name="ps", tag="psh")
                        e = expool.tile([128, 2 * SQC], BF, name="e", tag="ex")
                        for h in range(2):
                            pr = slice(h * 64, (h + 1) * 64)
                            nc.tensor.matmul(
                                ps[:, h * SQC + lo : h * SQC + hi],
                                khT[p][pr, jw],
                                qhT[p][pr, iw],
                                start=True, stop=True,
                            )
                        ps3 = ps.rearrange("p (h c) -> p h c", h=2)
                        e3 = e.rearrange("p (h c) -> p h c", h=2)
                        nc.scalar.activation(
                            e3[:, :, lo:hi], ps3[:, :, lo:hi], AF.Exp,
                            scale=1.0 / np.sqrt(DK),
                        )
                        for c0, tidx in c["muls"]:
                            for h in range(2):
                                cw = slice(h * SQC + c0, h * SQC + c0 + MCH)
                                nc.vector.tensor_mul(
                                    e[:, cw], e[:, cw], msk_sb[tidx][:]
                                )
                        pend.append((jn, j, lo, hi, e))
                        if len(pend) > 2:
                            emit_av(pend.pop(0))
                    for av in pend:
                        emit_av(av)
                    # aoU evacuation (incl. denominator row 64) frees psO
                    # for the next pair; ACT gathers the denominator rows
                    # from SBUF (no DMA in the chain)
                    paoU = []
                    for h in range(2):
                        aoU = aoupool.tile([65, SQC], F32, name="aoU",
                                           tag="aoU")
                        nc.vector.tensor_copy(aoU[:], psO[h][:])
                        aoUs.append(aoU)
                        paoU.append(aoU)
                    if i < NI - 1:
                        for h in range(2):
                            idx = 2 * p + h
                            nc.scalar.copy(
                                den_t[32 * idx : 32 * idx + 1, :],
                                paoU[h][64:65, :],
                            )
                    else:
                        # last chunk: per-pair reciprocal so pair 0's chain
                        # hides under pair 1's attention instead of tailing
                        den_p = nrm.tile([33, SQC], F32, name="den_p",
                                         tag="den_p")
                        nc.gpsimd.memset(den_p[:], 1.0)
                        for h in range(2):
                            nc.scalar.copy(
                                den_p[32 * h : 32 * h + 1, :],
                                paoU[h][64:65, :],
                            )
                        rc_p = nrm.tile([33, SQC], F32, name="rc_p", tag="rc_p")
                        nc.vector.reciprocal_approx_fast(rc_p[:], den_p[:])
                        rcb_p = nrm.tile([33, SQC], BF, name="rcb_p",
                                         tag="rcb_p")
                        nc.vector.tensor_copy(rcb_p[:], rc_p[:])
                        rcbs.append(rcb_p)
                    # deferred broadcast+normalize of the previous chunk,
                    # hidden under this chunk's second pair
                    if p == 0 and pending is not None:
                        emit_bcast_norm(pending)
                if i == NI - 1:
                    rcb_info = [(rcbs[0], 0), (rcbs[0], 32),
                                (rcbs[1], 0), (rcbs[1], 32)]
                    return {"i": i, "aoT": aoT, "aoUs": aoUs,
                            "rcb_info": rcb_info}
                return {"i": i, "aoT": aoT, "aoUs": aoUs, "den_t": den_t,
                        "rcb_info": None}

            def emit_recip(state):
                """One reciprocal serves all 4 heads (rows {0,32,64,96}).
                Emitted after the previous chunk's y casts so the DVE
                serves those first; only needed at the next pair boundary."""
                den_t = state["den_t"]
                rc_t = nrm.tile([97, SQC], F32, name="rc_t", tag="rc_t")
                nc.vector.reciprocal_approx_fast(rc_t[:], den_t[:])
                rcb_t = nrm.tile([97, SQC], BF, name="rcb_t", tag="rcb_t")
                nc.vector.tensor_copy(rcb_t[:], rc_t[:])
                state["rcb_info"] = [(rcb_t, 0), (rcb_t, 32), (rcb_t, 64),
                                     (rcb_t, 96)]

            def emit_norm_wo(state):
                """Wo projection + Y write for a finished i (deferred one
                chunk so the PE never waits on the reciprocal chain)."""
                i, aoT = state["i"], state["aoT"]
                for m in range(4):
                    rw = slice(m * 128, (m + 1) * 128)
                    y_sb = ypool.tile([128, D], F16, name="y_sb", tag="y_sb")
                    for n in range(2):
                        pY = psh.tile([128, SQC], F32, name="pY", tag="psh")
                        for kc in range(NPAIR):
                            nc.tensor.matmul(
                                pY[:],
                                aoT[kc][:, rw],
                                wo_sb[:, kc * D + n * SQC : kc * D + (n + 1) * SQC],
                                start=(kc == 0),
                                stop=(kc == NPAIR - 1),
                            )
                        nc.vector.tensor_copy(
                            y_sb[:, n * SQC : (n + 1) * SQC], pY[:]
                        )
                    # full 2KB dram rows per DMA, alternating queues
                    ENG[m % 2].dma_start(
                        out=y_d[i * SQC + m * 128 : i * SQC + (m + 1) * 128, :],
                        in_=y_sb[:],
                    )

            pending = None
            for i in range(NI):
                st = emit_attention(i, pending)
                if pending is not None:
                    emit_norm_wo(pending)
                if i < NI - 1:
                    emit_recip(st)
                pending = st
            emit_bcast_norm(pending)
            emit_norm_wo(pending)

    nc.compile()
    return nc


def _cls_sig(cls):
    out = []
    for row in cls:
        for c in row:
            if c is None:
                out.append(None)
            else:
                out.append((c["lo"], c["hi"], tuple(c["muls"])))
    return tuple(out)


def kernel(q, k, v, Wq, bq, Wk, bk, Wv, bv, Wo, bo, mask):
    global LAST_EXEC_NS, LAST_RESULT
    from concourse.bass_utils import run_bass_kernel_spmd

    q = np.asarray(q, np.float32)
    k = np.asarray(k, np.float32)
    v = np.asarray(v, np.float32)
    mask_st = np.asarray(mask).reshape(S, S).astype(bool)

    cls, mtiles = _classify_mask(mask_st)
    with_bias = not (
        np.all(np.asarray(bq) == 0)
        and np.all(np.asarray(bk) == 0)
        and np.all(np.asarray(bv) == 0)
    )

    sig = (_cls_sig(cls), len(mtiles), with_bias)
    if sig not in _prog_cache:
        _prog_cache[sig] = _build(cls, len(mtiles), with_bias)
    nc = _prog_cache[sig]

    def pack_w(wt, gd):  # [nch*128, gd] -> [128, nch*gd]
        nch = wt.shape[0] // 128
        return np.ascontiguousarray(
            wt.reshape(nch, 128, gd).transpose(1, 0, 2).reshape(128, nch * gd)
        ).astype(_BF)

    def pack_x(xb):  # [S, D] -> [NI*D, SQC]  (xT column-granules)
        xT = xb.T  # [D, S]
        return np.ascontiguousarray(
            xT.reshape(D, NI, SQC).transpose(1, 0, 2).reshape(NI * D, SQC)
        ).astype(_BF)

    xq_p = [pack_x(q[b]) for b in range(B)]
    xk_p = [pack_x(k[b]) for b in range(B)]
    xv_p = [pack_x(v[b]) for b in range(B)]

    in_maps = []
    for c in range(NCORE):
        b, g = divmod(c, TPG)
        rows = slice(g * GD, (g + 1) * GD)
        im = {
            "XQ": xq_p[b],
            "XK": xk_p[b],
            "XV": xv_p[b],
            "WQ": pack_w(np.ascontiguousarray(Wq[rows, :].T), GD),
            "WK": pack_w(np.ascontiguousarray(Wk[rows, :].T), GD),
            "WV": pack_w(np.ascontiguousarray(Wv[rows, :].T), GD),
            "WO": pack_w(np.ascontiguousarray(Wo[:, rows].T), D),
        }
        if mtiles:
            im["MSK"] = np.stack(mtiles)
        if with_bias:
            im["BQ"] = np.asarray(bq)[rows].reshape(1, GD).astype(_BF)
            im["BK"] = np.asarray(bk)[rows].reshape(1, GD).astype(_BF)
            im["BV"] = np.asarray(bv)[rows].reshape(1, GD).astype(_BF)
        in_maps.append(im)

    res = run_bass_kernel_spmd(nc, in_maps, list(range(NCORE)), trace=TRACE)
    LAST_RESULT = res
    LAST_EXEC_NS = res.exec_time_ns

    out = np.zeros((B, S, D), np.float32)
    for c in range(NCORE):
        out[c // TPG] += res.results[c]["Y"].astype(np.float32)
    out += np.asarray(bo, np.float32)
    return out


# revision 26
# speedup vs baseline: 1.0342x; 1.0013x over previous
"""Multi-head attention (B=2, S=2048, D=1024, H=16) on 8 TRN2 NeuronCores.

Sharding: 2-way data parallel over batch x 4-way tensor parallel over heads
(4 heads = 256 dims per core).  Each core computes, for its (batch, head
group): Q/K/V projections, causal attention, and a partial output
projection (row-sharded Wo).  The host sums the 4 fp16 partials per batch
and adds bo.

Schedule notes (built from neuron-profile traces of each revision; the
baseline this evolved from ran 210us, this version ~195us):
  - The HAM power governor starts the clocks at 1.2GHz and claws back
    full-rate (2.4GHz) windows in proportion to total engine activity,
    so the design minimizes total engine busy-time as much as stalls:
    merged exp instructions, few large DMAs, approximate reciprocal.
  - Inputs are repacked on the host into [Sq-chunk, D, 512] granules so
    the DMA stream (striped across the sync+gpsimd queues) arrives in
    exactly the order the projection matmuls consume it, one 128KB tile
    per contraction chunk.  The projection loop is q-block/k-block/
    v-block per Sq chunk; the phase is PE-bound, never DMA-starved.
  - Attention is co-limited by the PE and the ACT engine's exp (their
    per-chunk work is almost exactly equal).  scoresT[t, s] = kh @ qh^T
    per (pair, head) in one [128, 2*512] PSUM tile; one exp instruction
    covers both heads; AV matmuls are emitted TWO j-iterations behind
    their scores matmuls so the in-order PE rides out exp latency.
    PSUM budget: 3x2-bank scores bufs shared with the Wo-stage psB/pY
    tiles + 2 banks psO = all 8 banks.
  - Softmax: the AV matmul's ones-column gives the denominator in psO
    row 64; it is evacuated with the attention numerators in one DVE
    copy, ACT gathers the 4 rows onto partitions {0,32,64,96} of one
    tile (no DMA in the chain), one reciprocal_approx_fast serves all
    4 heads, and K=1 bf16 matmuls broadcast each reciprocal row across
    64 partitions; the normalize multiply reads that PSUM tile
    directly.  The chain is deferred one chunk: its broadcast+normalize
    hides under the next chunk's second pair, the Wo projection runs
    after that chunk, and the reciprocal is emitted behind the Y casts
    so the DVE serves the Wo stage first.  The last chunk uses per-pair
    chains to shorten the tail; the ACT exp table is pre-warmed during
    the projections.
  - Y partials are written as fp16 (halves the output traffic; the host
    accumulates in fp32; fp16 keeps 10 mantissa bits so the partial-sum
    rounding is negligible).
"""

import sys

sys.path.insert(0, "/opt/trn_rl_repo")

from contextlib import ExitStack

import ml_dtypes
import numpy as np

B, S, D, H = 2, 2048, 1024, 16
DK = D // H            # 64
NCORE = 8
DPB = 2                # data-parallel ways (batch)
TPG = NCORE // DPB     # 4 head groups
GH = H // TPG          # 4 heads per group
GD = GH * DK           # 256 dims per group
NPAIR = GH // 2        # 2 head pairs per group
SQC = 512              # Sq chunk (matmul moving dim)
SKC = 128              # Skv chunk (matmul partition dim)
MCH = 128              # mask chunk width
NI = S // SQC          # 4
NJ = S // SKC          # 16
KCH = D // 128         # 8 contraction chunks for the projections
HGD = KCH // 2 * GD    # weight-half column count (1024)

TRACE = False
LAST_EXEC_NS = None
LAST_RESULT = None

_BF = ml_dtypes.bfloat16
_prog_cache = {}


def _classify_mask(mask_st):
    """mask_st: [S, S] bool indexed [query s, key t].

    Returns (cls, tiles): cls[i][j] is None (skip) or a dict with
      lo, hi : active scoresT column range (multiples of MCH)
      muls   : list of (col_off, tile_idx) 128-col multiplicative masks
    tiles: deduped bf16 [SKC, MCH] tiles in scoresT orientation [t, s].
    """
    cls = [[None] * NJ for _ in range(NI)]
    tiles = []
    keys = {}

    def tile_idx(sub):
        t = np.ascontiguousarray(sub.T)  # [SKC t, MCH s]
        key = t.tobytes()
        if key not in keys:
            keys[key] = len(tiles)
            tiles.append(t.astype(_BF))
        return keys[key]

    for i in range(NI):
        sblk = mask_st[i * SQC : (i + 1) * SQC]
        for j in range(NJ):
            blk = sblk[:, j * SKC : (j + 1) * SKC]  # [SQC s, SKC t]
            any_col = blk.any(axis=1)               # per query col of scoresT
            if not any_col.any():
                continue
            nz = np.nonzero(any_col)[0]
            lo = (int(nz[0]) // MCH) * MCH
            hi = -(-(int(nz[-1]) + 1) // MCH) * MCH
            muls = []
            for c in range(lo, hi, MCH):
                sub = blk[c : c + MCH]              # [MCH s, SKC t]
                if not sub.all():
                    muls.append((c, tile_idx(sub)))
            cls[i][j] = {"lo": lo, "hi": hi, "muls": muls}
    return cls, tiles


def _build(cls, n_mask, with_bias):
    """Build the (SPMD, per-core) Bass program."""
    import concourse.bacc as bacc
    import concourse.tile as tile
    from concourse import mybir

    BF = mybir.dt.bfloat16
    F16 = mybir.dt.float16
    F32 = mybir.dt.float32
    AF = mybir.ActivationFunctionType

    nc = bacc.Bacc("TRN2", target_bir_lowering=False, debug=False)

    # x inputs repacked on host: [NI, D, SQC] -> rows sc*D+d, cols c
    xq_d = nc.dram_tensor("XQ", [NI * D, SQC], BF, kind="ExternalInput").ap()
    xk_d = nc.dram_tensor("XK", [NI * D, SQC], BF, kind="ExternalInput").ap()
    xv_d = nc.dram_tensor("XV", [NI * D, SQC], BF, kind="ExternalInput").ap()
    # packed weights: [128, KCH*GD], chunk kk at cols [kk*GD, (kk+1)*GD)
    wq_d = nc.dram_tensor("WQ", [128, KCH * GD], BF, kind="ExternalInput").ap()
    wk_d = nc.dram_tensor("WK", [128, KCH * GD], BF, kind="ExternalInput").ap()
    wv_d = nc.dram_tensor("WV", [128, KCH * GD], BF, kind="ExternalInput").ap()
    # packed Wo.T slice: [128, 2*D], chunk kc at cols [kc*D, (kc+1)*D)
    wo_d = nc.dram_tensor("WO", [128, 2 * D], BF, kind="ExternalInput").ap()
    msk_d = None
    if n_mask:
        msk_d = nc.dram_tensor(
            "MSK", [n_mask, SKC, MCH], BF, kind="ExternalInput"
        ).ap()
    if with_bias:
        bq_d = nc.dram_tensor("BQ", [1, GD], BF, kind="ExternalInput").ap()
        bk_d = nc.dram_tensor("BK", [1, GD], BF, kind="ExternalInput").ap()
        bv_d = nc.dram_tensor("BV", [1, GD], BF, kind="ExternalInput").ap()
    y_d = nc.dram_tensor("Y", [S, D], F16, kind="ExternalOutput").ap()

    with tile.TileContext(nc) as tc, ExitStack() as top:
        const = top.enter_context(tc.tile_pool(name="const", bufs=1))

        wq_sb = const.tile([128, KCH * GD], BF, name="wq_sb", tag="wq_sb")
        wk_sb = const.tile([128, KCH * GD], BF, name="wk_sb", tag="wk_sb")
        wv_sb = const.tile([128, KCH * GD], BF, name="wv_sb", tag="wv_sb")
        wo_sb = const.tile([128, 2 * D], BF, name="wo_sb", tag="wo_sb")
        # x tiles: xt[t][sc][kk] is [128, SQC] — one tile per contraction
        # chunk so each projection matmul waits only on its own 128KB DMA
        xt = {}
        for t in "qkv":
            xt[t] = [
                [
                    const.tile([128, SQC], BF, name=f"x{t}{sc}{kk}",
                               tag=f"x{t}{sc}{kk}")
                    for kk in range(KCH)
                ]
                for sc in range(NI)
            ]

        ENG = [nc.sync, nc.gpsimd]
        xdram = {"q": xq_d, "k": xk_d, "v": xv_d}

        def ldhalf(h, t, sc):
            # 4 contiguous [128, SQC] chunk transfers, one tile each
            for b in range(4):
                kk = h * 4 + b
                r0 = sc * D + kk * 128
                ENG[h].dma_start(
                    out=xt[t][sc][kk][:],
                    in_=xdram[t][r0 : r0 + 128, :],
                )

        # DMA stream order == projection consumption order, halves striped
        # across the two queues.
        for h in range(2):
            ENG[h].dma_start(
                out=wq_sb[:, h * HGD : (h + 1) * HGD],
                in_=wq_d[:, h * HGD : (h + 1) * HGD],
            )
        for h in range(2):
            ldhalf(h, "q", 0)
        for h in range(2):
            ENG[h].dma_start(
                out=wk_sb[:, h * HGD : (h + 1) * HGD],
                in_=wk_d[:, h * HGD : (h + 1) * HGD],
            )
        for h in range(2):
            ldhalf(h, "k", 0)
        for h in range(2):
            ENG[h].dma_start(
                out=wv_sb[:, h * HGD : (h + 1) * HGD],
                in_=wv_d[:, h * HGD : (h + 1) * HGD],
            )
        if with_bias:
            bq_sb = const.tile([1, GD], BF, name="bq_sb", tag="bq_sb")
            bk_sb = const.tile([1, GD], BF, name="bk_sb", tag="bk_sb")
            bv_sb = const.tile([1, GD], BF, name="bv_sb", tag="bv_sb")
            nc.sync.dma_start(out=bq_sb[:], in_=bq_d[:])
            nc.sync.dma_start(out=bk_sb[:], in_=bk_d[:])
            nc.sync.dma_start(out=bv_sb[:], in_=bv_d[:])
        for h in range(2):
            ldhalf(h, "v", 0)
        for sc in range(1, NI):
            for t in "qkv":
                for h in range(2):
                    ldhalf(h, t, sc)
            if sc == 1:
                msk_sb = []
                for mt in range(n_mask):
                    m = const.tile([SKC, MCH], BF, name=f"msk{mt}",
                                   tag=f"msk{mt}")
                    nc.sync.dma_start(out=m[:], in_=msk_d[mt])
                    msk_sb.append(m)
        if n_mask == 0:
            msk_sb = []
        for h in range(2):
            ENG[h].dma_start(
                out=wo_sb[:, h * D : (h + 1) * D],
                in_=wo_d[:, h * D : (h + 1) * D],
            )

        if with_bias:
            onesrow = const.tile([1, SQC], BF, name="onesrow", tag="onesrow")
            nc.vector.memset(onesrow[:], 1.0)

        # warm up the ACT exp table during the projection phase so the
        # 1.3us ACT_TABLE_LOAD isn't on the first attention exp
        warm0 = const.tile([1, 64], BF, name="warm0", tag="warm0")
        warm1 = const.tile([1, 64], BF, name="warm1", tag="warm1")
        nc.vector.memset(warm0[:], 0.0)
        nc.scalar.activation(warm1[:], warm0[:], AF.Exp)

        # persistent activations
        acts = top.enter_context(tc.tile_pool(name="acts", bufs=1))
        qhT = [acts.tile([128, S], BF, name=f"qhT{p}", tag=f"qhT{p}")
               for p in range(NPAIR)]
        khT = [acts.tile([128, S], BF, name=f"khT{p}", tag=f"khT{p}")
               for p in range(NPAIR)]
        # v in natural layout, 65 cols per head (64 dims + ones column)
        vh = [acts.tile([128, GH * 65], BF, name=f"vh{j}", tag=f"vh{j}")
              for j in range(NJ)]
        for j in range(NJ):
            v3 = vh[j].rearrange("p (h x) -> p h x", h=GH)
            nc.vector.memset(v3[:, :, 64:65], 1.0)

        def xsl(t, sc, kk):
            return xt[t][sc][kk][:]

        # ---------------- Phase B: projections ----------------
        with (
            tc.tile_pool(name="pproj", bufs=1, space="PSUM") as pproj,
        ):
            for sc in range(NI):
                psq = [pproj.tile([128, SQC], F32, name=f"psq{m}", tag=f"psq{m}")
                       for m in range(2)]
                psk = [pproj.tile([128, SQC], F32, name=f"psk{m}", tag=f"psk{m}")
                       for m in range(2)]
                psv = [pproj.tile([128, GD], F32, name=f"psv{m}", tag=f"psv{m}")
                       for m in range(4)]
                cc = slice(sc * SQC, (sc + 1) * SQC)
                # q block
                for kk in range(KCH):
                    st = kk == 0
                    sp = (kk == KCH - 1) and not with_bias
                    for m in range(2):
                        wcol = slice(kk * GD + m * 128, kk * GD + (m + 1) * 128)
                        nc.tensor.matmul(
                            psq[m][:], wq_sb[:, wcol], xsl("q", sc, kk),
                            start=st, stop=sp,
                        )
                if with_bias:
                    for m in range(2):
                        bcol = slice(m * 128, (m + 1) * 128)
                        nc.tensor.matmul(
                            psq[m][:], bq_sb[:, bcol], onesrow[:],
                            start=False, stop=True,
                        )
                for m in range(2):
                    nc.vector.tensor_copy(qhT[m][:, cc], psq[m][:])
                # k block
                for kk in range(KCH):
                    st = kk == 0
                    sp = (kk == KCH - 1) and not with_bias
                    for m in range(2):
                        wcol = slice(kk * GD + m * 128, kk * GD + (m + 1) * 128)
                        nc.tensor.matmul(
                            psk[m][:], wk_sb[:, wcol], xsl("k", sc, kk),
                            start=st, stop=sp,
                        )
                if with_bias:
                    for m in range(2):
                        bcol = slice(m * 128, (m + 1) * 128)
                        nc.tensor.matmul(
                            psk[m][:], bk_sb[:, bcol], onesrow[:],
                            start=False, stop=True,
                        )
                for m in range(2):
                    nc.vector.tensor_copy(khT[m][:, cc], psk[m][:])
                # v block
                for kk in range(KCH):
                    st = kk == 0
                    sp = (kk == KCH - 1) and not with_bias
                    for m in range(4):
                        nc.tensor.matmul(
                            psv[m][:],
                            xsl("v", sc, kk)[:, m * 128 : (m + 1) * 128],
                            wv_sb[:, kk * GD : (kk + 1) * GD],
                            start=st,
                            stop=sp,
                        )
                if with_bias:
                    for m in range(4):
                        nc.tensor.matmul(
                            psv[m][:], onesrow[:, 0:128], bv_sb[:],
                            start=False, stop=True,
                        )
                for m in range(4):
                    dst = vh[sc * 4 + m].rearrange("p (h x) -> p h x", h=GH)
                    src = psv[m].rearrange("p (h x) -> p h x", h=GH)
                    nc.vector.tensor_copy(dst[:, :, 0:64], src[:])

        # ---------------- Phase C: attention + Wo ----------------
        # ones on all 128 partitions; single rows are the lhsT of the K=1
        # denominator-broadcast matmuls (lhsT base must match rhs row base)
        onesP = const.tile([128, 64], BF, name="onesP", tag="onesP")
        nc.vector.memset(onesP[:], 1.0)

        with (
            # 3 two-bank bufs shared by the scores tiles and the Wo-stage
            # psB/pY tiles; +2 banks for psO = all 8 PSUM banks
            tc.tile_pool(name="psh", bufs=3, space="PSUM") as psh,
            tc.tile_pool(name="pso", bufs=1, space="PSUM") as pso,
            tc.tile_pool(name="ex", bufs=4) as expool,
            tc.tile_pool(name="nrm", bufs=2) as nrm,
            tc.tile_pool(name="aou", bufs=8) as aoupool,
            tc.tile_pool(name="ao", bufs=2) as aopool,
            tc.tile_pool(name="yout", bufs=3) as ypool,
        ):
            def emit_bcast_norm(state):
                """K=1 broadcast matmuls + normalize muls for a finished i;
                emitted between the two pairs of the NEXT chunk's attention
                so the Wo matmuls find aoT ready."""
                aoT, aoUs, rcb_info = (
                    state["aoT"], state["aoUs"], state["rcb_info"]
                )
                for idx in range(4):
                    rcb_t, r = rcb_info[idx]
                    p, h = divmod(idx, 2)
                    psB = psh.tile([64, SQC], F32, name="psB", tag="psh")
                    nc.tensor.matmul(
                        psB[:],
                        onesP[r : r + 1, :],
                        rcb_t[r : r + 1, :],
                        start=True, stop=True,
                        tile_position=(r, 0),
                    )
                    # normalize reads the PSUM broadcast tile directly
                    nc.vector.tensor_mul(
                        aoT[p][h * 64 : (h + 1) * 64, :],
                        aoUs[idx][0:64, :],
                        psB[:],
                    )

            def emit_attention(i, pending):
                """scores/exp/mask/AV + psO evacuation + reciprocal chain."""
                js = [j for j in range(NJ) if cls[i][j] is not None]
                assert js, "fully-masked query chunk not supported"
                aoT = [
                    aopool.tile([128, SQC], BF, name=f"aoT{p}", tag=f"aoT{p}")
                    for p in range(NPAIR)
                ]
                if i < NI - 1:
                    den_t = nrm.tile([97, SQC], F32, name="den_t", tag="den_t")
                    nc.gpsimd.memset(den_t[:], 1.0)
                aoUs = []
                rcbs = []
                for p in range(NPAIR):
                    psO = [
                        pso.tile([65, SQC], F32, name=f"psO{h}", tag=f"psO{h}")
                        for h in range(2)
                    ]

                    def emit_av(av):
                        jn, j, lo, hi, e = av
                        for h in range(2):
                            vcol = slice((2 * p + h) * 65, (2 * p + h + 1) * 65)
                            nc.tensor.matmul(
                                psO[h][:, lo:hi],
                                vh[j][:, vcol],
                                e[:, h * SQC + lo : h * SQC + hi],
                                start=(jn == 0), stop=(jn == len(js) - 1),
                            )

                    # AV matmuls are emitted TWO j-iterations behind the
                    # scores matmuls so the in-order PE never waits on the
                    # ACT engine's exp.
                    pend = []
                    for jn, j in enumerate(js):
                        c = cls[i][j]
                        lo, hi = c["lo"], c["hi"]
                        jw = slice(j * SKC, (j + 1) * SKC)
                        iw = slice(i * SQC + lo, i * SQC + hi)
                        # h0 in cols [0:SQC], h1 in cols [SQC:2*SQC]
                        ps = psh.tile([128, 2 * SQC], F32, name="ps", tag="psh")
                        e = expool.tile([128, 2 * SQC], BF, name="e", tag="ex")
                        for h in range(2):
                            pr = slice(h * 64, (h + 1) * 64)
                            nc.tensor.matmul(
                                ps[:, h * SQC + lo : h * SQC + hi],
                                khT[p][pr, jw],
                                qhT[p][pr, iw],
                                start=True, stop=True,
                            )
                        ps3 = ps.rearrange("p (h c) -> p h c", h=2)
                        e3 = e.rearrange("p (h c) -> p h c", h=2)
                        nc.scalar.activation(
                            e3[:, :, lo:hi], ps3[:, :, lo:hi], AF.Exp,
                            scale=1.0 / np.sqrt(DK),
                        )
                        for c0, tidx in c["muls"]:
                            for h in range(2):
                                cw = slice(h * SQC + c0, h * SQC + c0 + MCH)
                                nc.vector.tensor_mul(
                                    e[:, cw], e[:, cw], msk_sb[tidx][:]
                                )
                        pend.append((jn, j, lo, hi, e))
                        if len(pend) > 2:
                            emit_av(pend.pop(0))
                    for av in pend:
                        emit_av(av)
                    # aoU evacuation (incl. denominator row 64) frees psO
                    # for the next pair; ACT gathers the denominator rows
                    # from SBUF (no DMA in the chain)
                    paoU = []
                    for h in range(2):
                        aoU = aoupool.tile([65, SQC], F32, name="aoU",
                                           tag="aoU")
                        nc.vector.tensor_copy(aoU[:], psO[h][:])
                        aoUs.append(aoU)
                        paoU.append(aoU)
                    if i < NI - 1:
                        for h in range(2):
                            idx = 2 * p + h
                            nc.scalar.copy(
                                den_t[32 * idx : 32 * idx + 1, :],
                                paoU[h][64:65, :],
                            )
                    else:
                        # last chunk: per-pair reciprocal so pair 0's chain
                        # hides under pair 1's attention instead of tailing
                        den_p = nrm.tile([33, SQC], F32, name="den_p",
                                         tag="den_p")
                        nc.gpsimd.memset(den_p[:], 1.0)
                        for h in range(2):
                            nc.scalar.copy(
                                den_p[32 * h : 32 * h + 1, :],
                                paoU[h][64:65, :],
                            )
                        rc_p = nrm.tile([33, SQC], F32, name="rc_p", tag="rc_p")
                        nc.vector.reciprocal_approx_fast(rc_p[:], den_p[:])
                        rcb_p = nrm.tile([33, SQC], BF, name="rcb_p",
                                         tag="rcb_p")
                        nc.vector.tensor_copy(rcb_p[:], rc_p[:])
                        rcbs.append(rcb_p)
                    # deferred broadcast+normalize of the previous chunk,
                    # hidden under this chunk's second pair
                    if p == 0 and pending is not None:
                        emit_bcast_norm(pending)
                if i == NI - 1:
                    rcb_info = [(rcbs[0], 0), (rcbs[0], 32),
                                (rcbs[1], 0), (rcbs[1], 32)]
                    return {"i": i, "aoT": aoT, "aoUs": aoUs,
                            "rcb_info": rcb_info}
                return {"i": i, "aoT": aoT, "aoUs": aoUs, "den_t": den_t,
                        "rcb_info": None}

            def emit_recip(state):
                """One reciprocal serves all 4 heads (rows {0,32,64,96}).
                Emitted after the previous chunk's y casts so the DVE
                serves those first; only needed at the next pair boundary."""
                den_t = state["den_t"]
                rc_t = nrm.tile([97, SQC], F32, name="rc_t", tag="rc_t")
                nc.vector.reciprocal_approx_fast(rc_t[:], den_t[:])
                rcb_t = nrm.tile([97, SQC], BF, name="rcb_t", tag="rcb_t")
                nc.vector.tensor_copy(rcb_t[:], rc_t[:])
                state["rcb_info"] = [(rcb_t, 0), (rcb_t, 32), (rcb_t, 64),
                                     (rcb_t, 96)]

            def emit_norm_wo(state):
                """Wo projection + Y write for a finished i (deferred one
                chunk so the PE never waits on the reciprocal chain)."""
                i, aoT = state["i"], state["aoT"]
                for m in range(4):
                    rw = slice(m * 128, (m + 1) * 128)
                    y_sb = ypool.tile([128, D], F16, name="y_sb", tag="y_sb")
                    # kc-outer interleave: the kc=1 accumulates land two
                    # matmuls after their kc=0 starts, giving the DVE
                    # normalize of pair 1 extra headroom
                    pYs = [
                        psh.tile([128, SQC], F32, name="pY", tag="psh")
                        for n in range(2)
                    ]
                    for kc in range(NPAIR):
                        for n in range(2):
                            nc.tensor.matmul(
                                pYs[n][:],
                                aoT[kc][:, rw],
                                wo_sb[:, kc * D + n * SQC : kc * D + (n + 1) * SQC],
                                start=(kc == 0),
                                stop=(kc == NPAIR - 1),
                            )
                    for n in range(2):
                        nc.vector.tensor_copy(
                            y_sb[:, n * SQC : (n + 1) * SQC], pYs[n][:]
                        )
                    # full 2KB dram rows per DMA, alternating queues
                    ENG[m % 2].dma_start(
                        out=y_d[i * SQC + m * 128 : i * SQC + (m + 1) * 128, :],
                        in_=y_sb[:],
                    )

            pending = None
            for i in range(NI):
                st = emit_attention(i, pending)
                if pending is not None:
                    emit_norm_wo(pending)
                if i < NI - 1:
                    emit_recip(st)
                pending = st
            emit_bcast_norm(pending)
            emit_norm_wo(pending)

    nc.compile()
    return nc


def _cls_sig(cls):
    out = []
    for row in cls:
        for c in row:
            if c is None:
                out.append(None)
            else:
                out.append((c["lo"], c["hi"], tuple(c["muls"])))
    return tuple(out)


def kernel(q, k, v, Wq, bq, Wk, bk, Wv, bv, Wo, bo, mask):
    global LAST_EXEC_NS, LAST_RESULT
    from concourse.bass_utils import run_bass_kernel_spmd

    q = np.asarray(q, np.float32)
    k = np.asarray(k, np.float32)
    v = np.asarray(v, np.float32)
    mask_st = np.asarray(mask).reshape(S, S).astype(bool)

    cls, mtiles = _classify_mask(mask_st)
    with_bias = not (
        np.all(np.asarray(bq) == 0)
        and np.all(np.asarray(bk) == 0)
        and np.all(np.asarray(bv) == 0)
    )

    sig = (_cls_sig(cls), len(mtiles), with_bias)
    if sig not in _prog_cache:
        _prog_cache[sig] = _build(cls, len(mtiles), with_bias)
    nc = _prog_cache[sig]

    def pack_w(wt, gd):  # [nch*128, gd] -> [128, nch*gd]
        nch = wt.shape[0] // 128
        return np.ascontiguousarray(
            wt.reshape(nch, 128, gd).transpose(1, 0, 2).reshape(128, nch * gd)
        ).astype(_BF)

    def pack_x(xb):  # [S, D] -> [NI*D, SQC]  (xT column-granules)
        xT = xb.T  # [D, S]
        return np.ascontiguousarray(
            xT.reshape(D, NI, SQC).transpose(1, 0, 2).reshape(NI * D, SQC)
        ).astype(_BF)

    xq_p = [pack_x(q[b]) for b in range(B)]
    xk_p = [pack_x(k[b]) for b in range(B)]
    xv_p = [pack_x(v[b]) for b in range(B)]

    in_maps = []
    for c in range(NCORE):
        b, g = divmod(c, TPG)
        rows = slice(g * GD, (g + 1) * GD)
        im = {
            "XQ": xq_p[b],
            "XK": xk_p[b],
            "XV": xv_p[b],
            "WQ": pack_w(np.ascontiguousarray(Wq[rows, :].T), GD),
            "WK": pack_w(np.ascontiguousarray(Wk[rows, :].T), GD),
            "WV": pack_w(np.ascontiguousarray(Wv[rows, :].T), GD),
            "WO": pack_w(np.ascontiguousarray(Wo[:, rows].T), D),
        }
        if mtiles:
            im["MSK"] = np.stack(mtiles)
        if with_bias:
            im["BQ"] = np.asarray(bq)[rows].reshape(1, GD).astype(_BF)
            im["BK"] = np.asarray(bk)[rows].reshape(1, GD).astype(_BF)
            im["BV"] = np.asarray(bv)[rows].reshape(1, GD).astype(_BF)
        in_maps.append(im)

    res = run_bass_kernel_spmd(nc, in_maps, list(range(NCORE)), trace=TRACE)
    LAST_RESULT = res
    LAST_EXEC_NS = res.exec_time_ns

    out = np.zeros((B, S, D), np.float32)
    for c in range(NCORE):
        out[c // TPG] += res.results[c]["Y"].astype(np.float32)
    out += np.asarray(bo, np.float32)
    return out
